# revision 13
# baseline (speedup 1.0000x reference)
"""HNLoRALinear Trainium2 kernel (bf16 v4, col-tiled LoRA).

out[b,s,o] = x[b] @ W^T + bias + SCALE * (x[b] @ A[b]) @ B[b]

Sharding: 8 cores = 4 batches x 2 sequence-halves. Each core computes
its [1024 tokens, 4096 outs] output block, TRANSPOSED on device
(outs on PSUM partitions, tokens as the moving dim). All inputs are
bf16 (PE streams 1 col/cycle for bf16 and f32r alike, but bf16 halves
DMA bytes, halves SBUF, and enables Fast Weight Load so the 128-col
LDWEIGHTS fully hides under the 512-col stream); accumulation is fp32
in PSUM.

Startup: 10 dummy matmuls on a memset scratch tile warm the PE
p-state while the first DMAs land, then o=0/o=1's base matmuls run on
the per-x-group DMA cadence so the x-load window retires real work
(4 open PSUM groups + 2 lora banks + 1 scratch = 7 of 8 banks).

The x@A low-rank pass uses PE column tiling: the [128,16] A-chunk
stationaries occupy one 32-col tile each, so 4 consecutive k-chunks
stream CONCURRENTLY on 4 XBUSes into 4 PSUM quadrants (16 quad-spans
instead of 64 serial 512-col streams). The 4 quadrant partials are
then summed on the vector engine into the bf16 `low` tile.

The LoRA correction + bias ride along as one extra matmul per output
tile, with the stationary padded to the full K=128 (rows 0..15 =
SCALE*B chunk, row 16 = bias, rest 0) so the LDWEIGHTS pull-ahead
pipeline stays intact across the tile boundary.
"""
import numpy as np
import ml_dtypes

import concourse.bass as bass  # noqa: F401  (bass must import before tile)
import concourse.mybir as mybir
import concourse.tile as tile
from concourse import bacc
from concourse.bass_utils import run_bass_kernel_spmd

# Problem shapes (hardcoded per contract).
B, S, D_IN, D_OUT, R = 4, 2048, 4096, 4096, 16
XG = 8                 # x DMA groups (separate tiles so deps are per-group)
SCALE = 32.0 / 16.0
SH = S // 2            # tokens per core
P = 128
KC = D_IN // P         # 32 contraction chunks
O_CHUNKS = D_OUT // P  # 32 output-feature chunks (PSUM partition dim)
TN = 512               # moving-dim token group width
TGROUPS = SH // TN     # 2
KG = KC // XG          # k-chunks per x group
RA = R + 1             # augmented rank (lora + bias row)
N_WARM = 22            # PE warm-up dummy matmuls
QT = 4                 # col-tiles per quad (32-col tiles)

BF16 = ml_dtypes.bfloat16

_cached_nc = None


def _build():
    bf16 = mybir.dt.bfloat16
    f32 = mybir.dt.float32
    nc = bacc.Bacc(
        "TRN2", target_bir_lowering=False, debug=False, enable_asserts=False
    )
    xt = nc.dram_tensor("xt", [XG, P, KG * SH], bf16, kind="ExternalInput")
    wt = nc.dram_tensor("wt", [O_CHUNKS, P, KC * P], bf16, kind="ExternalInput")
    apk = nc.dram_tensor("apack", [P, KC * R], bf16, kind="ExternalInput")
    bga = nc.dram_tensor("baug", [P, D_OUT], bf16, kind="ExternalInput")
    ot_d = nc.dram_tensor("ot", [D_OUT, SH], f32, kind="ExternalOutput")

    with tile.TileContext(nc) as tc:
        with (
            tc.tile_pool(name="xp", bufs=1) as xp,
            tc.tile_pool(name="wp", bufs=5) as wp,
            tc.tile_pool(name="cp", bufs=1) as cp,
            tc.tile_pool(name="op", bufs=3) as op,
            tc.tile_pool(name="pp", bufs=5, space="PSUM") as pp,
            tc.tile_pool(name="lp", bufs=2, space="PSUM") as lp,
            tc.tile_pool(name="sp", bufs=1, space="PSUM") as sp,
        ):
            at = cp.tile([P, KC * R], bf16, name="at")
            nc.sync.dma_start(out=at[:], in_=apk.ap())

            def load_w_strip(o):
                # One fully-contiguous 2D DMA per strip (host pre-packs W
                # as [o_chunk, partition, k*128+c]) -- 3D patterns cost one
                # DMA descriptor per (partition, k) and dispatch ~20x slower.
                wk = wp.tile([P, KC * P], bf16, name="wk")
                nc.sync.dma_start(out=wk[:], in_=wt.ap()[o])
                return wk

            # x^T fully resident as XG separate [128, KG, 1024] tiles (one
            # DMA each) so each matmul depends only on the chunk-group it
            # reads. Dispatch order is tuned so phase 1 (o=0/o=1 base
            # matmuls, ~3.5us of PE work per x group) never starves on the
            # input queue: wk0/wk1 right after their first consumers' x
            # groups, bt (only needed by the augmented matmuls ~40us in)
            # after xg3.
            xgs = [None] * XG
            w_strips = {}
            bt = cp.tile([P, D_OUT], bf16, name="bt")

            def load_xg(g):
                xg = xp.tile([P, KG * SH], bf16, name=f"xg{g}", tag=f"xg{g}")
                nc.sync.dma_start(out=xg[:], in_=xt.ap()[g])
                xgs[g] = xg

            w_strips[0] = load_w_strip(0)
            load_xg(0)
            load_xg(1)
            w_strips[1] = load_w_strip(1)
            load_xg(2)
            load_xg(3)
            nc.sync.dma_start(out=bt[:], in_=bga.ap())
            for g in range(4, XG):
                load_xg(g)
            # Preload the first steady-loop W strips here: their sync-engine
            # dispatches must precede the o0/o1 output-DMA dispatches, which
            # block the sync queue on the (late) augmented-matmul copies.
            for o in range(2, 5):
                w_strips[o] = load_w_strip(o)

            def xsl(k, t):
                return xgs[k // KG][:, (k % KG) * SH + t * TN : (k % KG) * SH + (t + 1) * TN]

            # PE warm-up: dummy matmuls on a zeroed scratch tile with no DMA
            # dependency, so the PE ramps to full clock while at/xg0 land.
            scr = cp.tile([P, TN], bf16, name="scr")
            nc.gpsimd.memset(scr[:], 0.0)
            psd = sp.tile([P, TN], f32, name="psd")
            for _ in range(N_WARM):
                nc.tensor.matmul(psd[:], scr[:, 0:P], scr[:], start=True, stop=True)

            low = cp.tile([P, SH], bf16, name="low")
            nc.gpsimd.memset(low[:], 1.0)

            # Phase 1: o=0 and o=1 base matmuls ride the per-x-group DMA
            # cadence (4 PSUM groups stay open until their augmented matmul).
            ps_early = [
                [pp.tile([P, TN], f32, name="ps") for _ in range(TGROUPS)]
                for _ in range(2)
            ]
            for g in range(XG):
                for o in range(2):
                    for t in range(TGROUPS):
                        for k in range(g * KG, (g + 1) * KG):
                            nc.tensor.matmul(
                                ps_early[o][t][:],
                                w_strips[o][:, k * P : (k + 1) * P],
                                xsl(k, t),
                                start=(k == 0),
                                stop=False,
                            )

            # Phase 2: x@A via column tiling -- k-chunks k=4q+j stream
            # concurrently into col-tile j / PSUM quadrant j.
            pls = [lp.tile([P, TN], f32, name="pl") for _ in range(TGROUPS)]
            for t in range(TGROUPS):
                for q in range(KC // QT):
                    for j in range(QT):
                        k = QT * q + j
                        nc.tensor.matmul(
                            pls[t][32 * j : 32 * j + R, :],
                            at[:, k * R : (k + 1) * R],
                            xsl(k, t),
                            start=(q == 0),
                            stop=(q == KC // QT - 1),
                            tile_position=(0, 32 * j),
                        )

            # Phase 3: copy each quadrant partial to the SAME partitions of
            # `low` (rows 32j..32j+15). No cross-partition reduce is needed:
            # bt carries SCALE*B at those same four row positions, so the
            # augmented matmul's K=128 contraction sums the quadrants.
            for t in range(TGROUPS):
                sl = slice(t * TN, (t + 1) * TN)
                for j in range(QT):
                    nc.vector.tensor_copy(
                        low[32 * j : 32 * j + R, sl], pls[t][32 * j : 32 * j + R, :]
                    )

            # Phase 4: augmented matmuls close the early groups; then the
            # steady o-loop.
            def finish_tile(o, ps, otile, t):
                nc.tensor.matmul(
                    ps[:],
                    bt[:, o * P : (o + 1) * P],
                    low[:, t * TN : (t + 1) * TN],
                    start=False,
                    stop=True,
                )
                nc.vector.tensor_copy(otile[:, t * TN : (t + 1) * TN], ps[:])
                nc.sync.dma_start(
                    out=ot_d.ap()[o * P : (o + 1) * P, t * TN : (t + 1) * TN],
                    in_=otile[:, t * TN : (t + 1) * TN],
                )

            early_otiles = [op.tile([P, SH], f32, name="otile") for _ in range(2)]
            for o in range(2):
                for t in range(TGROUPS):
                    finish_tile(o, ps_early[o][t], early_otiles[o], t)

            for o in range(2, O_CHUNKS):
                wk = w_strips.pop(o) if o in w_strips else load_w_strip(o)
                if o + 1 < O_CHUNKS and (o + 1) not in w_strips:
                    w_strips[o + 1] = load_w_strip(o + 1)
                otile = op.tile([P, SH], f32, name="otile")
                for t in range(TGROUPS):
                    ps = pp.tile([P, TN], f32, name="ps")
                    for k in range(KC):
                        nc.tensor.matmul(
                            ps[:],
                            wk[:, k * P : (k + 1) * P],
                            xsl(k, t),
                            start=(k == 0),
                            stop=False,
                        )
                    finish_tile(o, ps, otile, t)
    nc.compile()
    return nc


def _get_nc():
    global _cached_nc
    if _cached_nc is None:
        _cached_nc = _build()
    return _cached_nc


def _in_maps(x, weight, bias, lora_A, lora_B):
    # W^T packed as [o_chunk, partition, k*128+c]: element (o*128+c, k*128+p)
    # of W -> wt[o, p, k*128+c]; shared by all cores.
    wt = np.ascontiguousarray(
        weight.T.reshape(KC, P, O_CHUNKS, P).transpose(2, 1, 0, 3).reshape(
            O_CHUNKS, P, KC * P
        )
    ).astype(BF16)
    maps = []
    for c in range(8):
        b, h = divmod(c, 2)
        xtc = np.ascontiguousarray(
            x[b, h * SH : (h + 1) * SH, :].T.reshape(XG, KG, P, SH)
            .transpose(0, 2, 1, 3)
            .reshape(XG, P, KG * SH)
        ).astype(BF16)
        apk = np.ascontiguousarray(
            lora_A[b].reshape(KC, P, R).transpose(1, 0, 2).reshape(P, KC * R)
        ).astype(BF16)
        baug = np.zeros((P, D_OUT), np.float32)
        for j in range(4):
            baug[32 * j : 32 * j + R] = lora_B[b] * SCALE
        baug[R] = bias
        maps.append({"xt": xtc, "wt": wt, "apack": apk, "baug": baug.astype(BF16)})
    return maps


def kernel(x, weight, bias, lora_A, lora_B, _trace=False, _tmpdir=None):
    x = np.asarray(x, dtype=np.float32)
    weight = np.asarray(weight, dtype=np.float32)
    bias = np.asarray(bias, dtype=np.float32)
    lora_A = np.asarray(lora_A, dtype=np.float32)
    lora_B = np.asarray(lora_B, dtype=np.float32)

    nc = _get_nc()
    maps = _in_maps(x, weight, bias, lora_A, lora_B)
    res = run_bass_kernel_spmd(
        nc, maps, list(range(8)), trace=_trace, tmpdir=_tmpdir
    )
    out = np.empty((B, S, D_OUT), np.float32)
    for c in range(8):
        b, h = divmod(c, 2)
        out[b, h * SH : (h + 1) * SH, :] = res.results[c]["ot"].T
    if _trace:
        return out, res
    return out
